# revision 22
# baseline (speedup 1.0000x reference)
"""Trainium2 Bass kernel for nn_MoEScoreHead (moe_routing).

Computes, for x [B=16384, H=2048]:
    logits = x @ Wg + bg                      [B, 8]
    top2 -> softmax -> dense combine weights  [B, 8]
    h = gelu(x @ W1[e] + b1[e])               [B, 8, 512]
    o[b,e] = h[b,e,:] @ W2[e]                 [B, 8]
    final_scores = sum(combine * o, -1)       [B, 1]
Returns (final_scores, logits) like the reference.

Sharding: data-parallel over batch across 8 NeuronCores (2048 tokens/core),
expert/gate weights replicated.

Per-core dataflow:
  - x shard is streamed in b-tiles of 128 tokens, transposed on the PE
    (via identity matmul) and kept resident in SBUF as bf16 xT tiles
    [h-part, b] (8MB).
  - Gating logits per b-tile via 16 accumulating matmuls (lhsT=xT, rhs=Wg).
  - W1 is streamed per expert-group (2 experts) as f32 and cast to bf16
    (double-buffered), then for each b-tile 16x2 accumulating matmuls
    produce psum [128, 512] per expert; ACT applies exact GELU; DVE does a
    fused multiply(+W2)-reduce to get o[:, e].
  - Top-2 softmax combine computed with DVE/ACT ops on [128, 8] tiles
    (max, second max via masking, exp, masked normalize).
"""

import os
import sys

import numpy as np

for _p in ("/opt/trn_rl_repo", "/root/.axon_site/_ro/trn_rl_repo"):
    if os.path.isdir(_p) and _p not in sys.path:
        sys.path.insert(0, _p)

from contextlib import ExitStack

import concourse.bass as bass
import concourse.bacc as bacc
import concourse.mybir as mybir
import concourse.tile as tile
from concourse.bass_utils import run_bass_kernel_spmd
from concourse.masks import make_identity

B, H, E, TOPK, HF = 16384, 2048, 8, 2, 512
NCORES = 8
P = 128
NH = H // P            # 16 h-tiles
EG = 2                 # experts per group
NG = E // EG           # 4 groups
F32 = mybir.dt.float32
MDT = mybir.dt.bfloat16
GELU = mybir.ActivationFunctionType.Gelu
EXP = mybir.ActivationFunctionType.Exp
ALU = mybir.AluOpType
AXX = mybir.AxisListType.X
NEG_BIG = -1e30


def _emit(ctx, tc, aps, nb, has_b1, has_bg, act_fn=GELU, variant='full'):
    nc = tc.nc
    x, W1, b1, W2, Wg, bg, scores, logits = aps

    const = ctx.enter_context(tc.tile_pool(name="const", bufs=1))
    xstage = ctx.enter_context(tc.tile_pool(name="xstage", bufs=2))
    xtp = ctx.enter_context(tc.tile_pool(name="xtp", bufs=1))
    w1stage = ctx.enter_context(tc.tile_pool(name="w1stage", bufs=2))
    w1bfp = ctx.enter_context(tc.tile_pool(name="w1bfp", bufs=2))
    gelup = ctx.enter_context(tc.tile_pool(name="gelup", bufs=4))
    small = ctx.enter_context(tc.tile_pool(name="small", bufs=2))
    persist = ctx.enter_context(tc.tile_pool(name="persist", bufs=1))
    tp_psum = ctx.enter_context(tc.tile_pool(name="tp_psum", bufs=2, space="PSUM"))
    g_psum = ctx.enter_context(tc.tile_pool(name="g_psum", bufs=2, space="PSUM"))
    mm_psum = ctx.enter_context(tc.tile_pool(name="mm_psum", bufs=4, space="PSUM"))

    # ---- constants ----
    ident = const.tile([P, P], F32)
    make_identity(nc, ident)
    # Warmup transpose: absorbs the identity-init (gpsimd) wait on the PE so
    # real transposes carry only their input-DMA wait (the transpose/LDW
    # instruction has a single sync-wait slot in walrus codegen).
    pt_warm = tp_psum.tile([P, P], F32, name="pt", tag="pt")
    nc.tensor.transpose(pt_warm, ident, ident)

    wg_f = const.tile([P, NH, E], F32)
    nc.sync.dma_start(wg_f, Wg.rearrange("(j p) e -> p j e", p=P))

    w2b = const.tile([P, E, HF], F32)
    nc.sync.dma_start(
        w2b, W2.rearrange("(o e) f -> o e f", o=1).partition_broadcast(P))

    if has_b1:
        b1b = const.tile([P, E, HF], F32)
        nc.sync.dma_start(
            b1b, b1.rearrange("(o e) f -> o e f", o=1).partition_broadcast(P))
    if has_bg:
        bgb = const.tile([P, E], F32)
        nc.sync.dma_start(
            bgb, bg.rearrange("(o e) -> o e", o=1).partition_broadcast(P))

    negbig = const.tile([P, E], F32)
    nc.vector.memset(negbig, NEG_BIG)

    # ---- persistent tiles ----
    xt = xtp.tile([P, NH, nb, P], MDT)        # x^T resident, bf16
    logit_sb = persist.tile([P, nb, E], F32)
    o_sb = persist.tile([P, nb, E], F32)
    fin_all = persist.tile([P, nb], F32)

    # ---- phase 1: stream x, transpose to xT, gating logits ----
    for b in range(nb):
        xn = xstage.tile([P, H], F32, name="xn")
        nc.sync.dma_start(xn, x[P * b:P * (b + 1), :])
        gp = g_psum.tile([P, E], F32, name="gp")
        xtfs = []
        for j in range(NH):
            pt = tp_psum.tile([P, P], F32, name="pt", tag="pt")
            nc.tensor.transpose(pt, xn[:, P * j:P * (j + 1)], ident)
            xtf = xstage.tile([P, P], F32, name="xtf", tag="xtf", bufs=NH + 2)
            nc.vector.tensor_copy(xtf, pt)
            nc.vector.tensor_copy(xt[:, j, b, :], xtf)
            xtfs.append(xtf)
        # fp32 gate matmuls, grouped after the transposes so the PE is not
        # ping-ponging with DVE evacuations: selection-critical logits need
        # full precision (bf16 logits flip the 2nd/3rd expert choice on
        # ~30/16k tokens).
        for j in range(NH):
            nc.tensor.matmul(gp, lhsT=xtfs[j], rhs=wg_f[:, j, :],
                             start=(j == 0), stop=(j == NH - 1))
        if has_bg:
            nc.vector.tensor_add(logit_sb[:, b, :], gp, bgb)
        else:
            nc.vector.tensor_copy(logit_sb[:, b, :], gp)

    if variant == "p1":
        nc.sync.dma_start(logits.rearrange("(bt p) e -> p bt e", p=P), logit_sb)
        return

    # ---- phase 2: experts, grouped ----
    for g in range(NG):
        w1bf = w1bfp.tile([P, NH, EG, HF], MDT, name="w1bf")
        for j in range(NH):
            st = w1stage.tile([P, EG, HF], F32, name="st")
            nc.sync.dma_start(
                st, W1[g * EG:(g + 1) * EG, P * j:P * (j + 1), :]
                .rearrange("e p f -> p e f"))
            nc.vector.tensor_copy(w1bf[:, j, :, :], st)
        for b in range(nb):
            pss = []
            for ee in range(EG):
                ps = mm_psum.tile([P, HF], F32, name="ps", tag="ps")
                pss.append(ps)
            for j in range(NH):
                for ee in range(EG):
                    nc.tensor.matmul(pss[ee], lhsT=xt[:, j, b, :],
                                     rhs=w1bf[:, j, ee, :],
                                     start=(j == 0), stop=(j == NH - 1))
            for ee in range(EG):
                e = g * EG + ee
                gl = gelup.tile([P, HF], F32, name="gl", tag="gl")
                if has_b1:
                    nc.vector.tensor_add(gl, pss[ee], b1b[:, e, :])
                    nc.scalar.activation(gl, gl, act_fn)
                else:
                    nc.scalar.activation(gl, pss[ee], act_fn)
                if variant == "p2a":
                    nc.vector.tensor_reduce(o_sb[:, b, e:e + 1], gl,
                                            axis=AXX, op=ALU.add)
                else:
                    prod2 = gelup.tile([P, HF], F32, name="prod2", tag="prod2")
                    nc.vector.tensor_mul(prod2, gl, w2b[:, e, :])
                    nc.vector.tensor_reduce(o_sb[:, b, e:e + 1], prod2,
                                            axis=AXX, op=ALU.add)

    if variant in ("p2", "p2a"):
        nc.sync.dma_start(logits.rearrange("(bt p) e -> p bt e", p=P), logit_sb)
        nc.sync.dma_start(scores.rearrange("(bt p) o -> p bt o", p=P),
                          o_sb[:, :, 0:1])
        return

    # ---- phase 3: top-2 softmax combine ----
    for b in range(nb):
        lg = logit_sb[:, b, :]
        m1 = small.tile([P, 1], F32, name="m1")
        nc.vector.tensor_reduce(m1, lg, axis=AXX, op=ALU.max)
        nm1 = small.tile([P, 1], F32, name="nm1")
        nc.vector.tensor_scalar_mul(nm1, m1, -1.0)
        eq = small.tile([P, E], mybir.dt.uint8, name="eq")
        nc.vector.tensor_scalar(eq, lg, m1, None, op0=ALU.is_equal)
        msk = small.tile([P, E], F32, name="msk")
        nc.vector.select(msk, eq, negbig, lg)
        m2 = small.tile([P, 1], F32, name="m2")
        nc.vector.tensor_reduce(m2, msk, axis=AXX, op=ALU.max)
        t = small.tile([P, E], F32, name="t")
        nc.scalar.activation(t, lg, EXP, bias=nm1, scale=1.0)
        keep = small.tile([P, E], F32, name="keep")
        nc.vector.tensor_scalar(keep, lg, m2, None, op0=ALU.is_ge)
        t2 = small.tile([P, E], F32, name="t2")
        nc.vector.tensor_mul(t2, t, keep)
        z = small.tile([P, 1], F32, name="z")
        nc.vector.tensor_reduce(z, t2, axis=AXX, op=ALU.add)
        s = small.tile([P, 1], F32, name="s")
        prod = small.tile([P, E], F32, name="prod")
        nc.vector.tensor_mul(prod, t2, o_sb[:, b, :])
        nc.vector.tensor_reduce(s, prod, axis=AXX, op=ALU.add)
        rz = small.tile([P, 1], F32, name="rz")
        nc.vector.reciprocal(rz, z)
        nc.vector.tensor_mul(fin_all[:, b:b + 1], s, rz)

    # ---- outputs ----
    nc.sync.dma_start(logits.rearrange("(bt p) e -> p bt e", p=P), logit_sb)
    nc.sync.dma_start(scores.rearrange("(bt p) o -> p bt o", p=P),
                      fin_all.rearrange("p (bt o) -> p bt o", o=1))


def build_nc(nb, has_b1, has_bg, act_fn=GELU, variant='full'):
    bl = nb * P
    nc = bacc.Bacc("TRN2", target_bir_lowering=False, debug=False)
    x = nc.declare_dram_parameter("x", [bl, H], F32, isOutput=False)
    W1 = nc.declare_dram_parameter("W1", [E, H, HF], F32, isOutput=False)
    b1 = nc.declare_dram_parameter("b1", [E, HF], F32, isOutput=False)
    W2 = nc.declare_dram_parameter("W2", [E, HF], F32, isOutput=False)
    Wg = nc.declare_dram_parameter("Wg", [H, E], F32, isOutput=False)
    bg = nc.declare_dram_parameter("bg", [E], F32, isOutput=False)
    scores = nc.declare_dram_parameter("scores", [bl, 1], F32, isOutput=True)
    logits = nc.declare_dram_parameter("logits", [bl, E], F32, isOutput=True)
    aps = (x[:, :], W1[:, :, :], b1[:, :], W2[:, :], Wg[:, :], bg[:],
           scores[:, :], logits[:, :])
    with tile.TileContext(nc) as tc:
        with ExitStack() as ctx:
            _emit(ctx, tc, aps, nb, has_b1, has_bg, act_fn, variant)
    nc.compile()
    return nc


_CACHE = {}


def _get_nc(nb, has_b1, has_bg):
    key = (nb, has_b1, has_bg)
    if key not in _CACHE:
        _CACHE[key] = build_nc(nb, has_b1, has_bg)
    return _CACHE[key]


def kernel(x, W1, b1, W2, Wg, bg, trace=False):
    x = np.ascontiguousarray(np.asarray(x, np.float32))
    W1 = np.ascontiguousarray(np.asarray(W1, np.float32))
    b1 = np.ascontiguousarray(np.asarray(b1, np.float32))
    W2 = np.ascontiguousarray(np.asarray(W2, np.float32))
    Wg = np.ascontiguousarray(np.asarray(Wg, np.float32))
    bg = np.ascontiguousarray(np.asarray(bg, np.float32))
    assert x.shape == (B, H)
    bl = B // NCORES
    nb = bl // P
    has_b1 = bool(np.any(b1))
    has_bg = bool(np.any(bg))
    nc = _get_nc(nb, has_b1, has_bg)
    in_maps = [
        {"x": x[c * bl:(c + 1) * bl], "W1": W1, "b1": b1, "W2": W2,
         "Wg": Wg, "bg": bg}
        for c in range(NCORES)
    ]
    res = run_bass_kernel_spmd(nc, in_maps, core_ids=list(range(NCORES)),
                               trace=trace)
    scores = np.concatenate([res.results[c]["scores"] for c in range(NCORES)],
                            axis=0)
    logits = np.concatenate([res.results[c]["logits"] for c in range(NCORES)],
                            axis=0)
    if trace:
        kernel.last_exec_time_ns = res.exec_time_ns
    return scores, logits


kernel.last_exec_time_ns = None


# revision 23
# speedup vs baseline: 1.0523x; 1.0523x over previous
"""Trainium2 Bass kernel for nn_MoEScoreHead (moe_routing).

Computes, for x [B=16384, H=2048]:
    logits = x @ Wg + bg                      [B, 8]
    top2 -> softmax -> dense combine weights  [B, 8]
    h = gelu(x @ W1[e] + b1[e])               [B, 8, 512]
    o[b,e] = h[b,e,:] @ W2[e]                 [B, 8]
    final_scores = sum(combine * o, -1)       [B, 1]
Returns (final_scores, logits) like the reference.

Sharding: data-parallel over batch across 8 NeuronCores (2048 tokens/core),
expert/gate weights replicated.

Per-core dataflow:
  - x shard is streamed in b-tiles of 128 tokens, transposed on the PE
    (via identity matmul) and kept resident in SBUF as bf16 xT tiles
    [h-part, b] (8MB).
  - Gating logits per b-tile via 16 accumulating matmuls (lhsT=xT, rhs=Wg).
  - W1 is streamed per expert-group (2 experts) as f32 and cast to bf16
    (double-buffered), then for each b-tile 16x2 accumulating matmuls
    produce psum [128, 512] per expert; ACT applies exact GELU; DVE does a
    fused multiply(+W2)-reduce to get o[:, e].
  - Top-2 softmax combine computed with DVE/ACT ops on [128, 8] tiles
    (max, second max via masking, exp, masked normalize).
"""

import os
import sys

import numpy as np

for _p in ("/opt/trn_rl_repo", "/root/.axon_site/_ro/trn_rl_repo"):
    if os.path.isdir(_p) and _p not in sys.path:
        sys.path.insert(0, _p)

from contextlib import ExitStack

import concourse.bass as bass
import concourse.bacc as bacc
import concourse.mybir as mybir
import concourse.tile as tile
from concourse.bass_utils import run_bass_kernel_spmd
from concourse.masks import make_identity

B, H, E, TOPK, HF = 16384, 2048, 8, 2, 512
NCORES = 8
P = 128
NH = H // P            # 16 h-tiles
EG = 2                 # experts per group
NG = E // EG           # 4 groups
F32 = mybir.dt.float32
MDT = mybir.dt.bfloat16
GELU = mybir.ActivationFunctionType.Gelu
EXP = mybir.ActivationFunctionType.Exp
ALU = mybir.AluOpType
AXX = mybir.AxisListType.X
NEG_BIG = -1e30


def _emit(ctx, tc, aps, nb, has_b1, has_bg, act_fn=GELU, variant='full'):
    nc = tc.nc
    x, W1, b1, W2, Wg, bg, scores, logits = aps

    const = ctx.enter_context(tc.tile_pool(name="const", bufs=1))
    xstage = ctx.enter_context(tc.tile_pool(name="xstage", bufs=2))
    xtp = ctx.enter_context(tc.tile_pool(name="xtp", bufs=1))
    w1stage = ctx.enter_context(tc.tile_pool(name="w1stage", bufs=2))
    w1bfp = ctx.enter_context(tc.tile_pool(name="w1bfp", bufs=2))
    gelup = ctx.enter_context(tc.tile_pool(name="gelup", bufs=4))
    small = ctx.enter_context(tc.tile_pool(name="small", bufs=2))
    persist = ctx.enter_context(tc.tile_pool(name="persist", bufs=1))
    tp_psum = ctx.enter_context(tc.tile_pool(name="tp_psum", bufs=2, space="PSUM"))
    g_psum = ctx.enter_context(tc.tile_pool(name="g_psum", bufs=2, space="PSUM"))
    mm_psum = ctx.enter_context(tc.tile_pool(name="mm_psum", bufs=4, space="PSUM"))

    # ---- constants ----
    ident = const.tile([P, P], F32)
    make_identity(nc, ident)
    # Warmup transpose: absorbs the identity-init (gpsimd) wait on the PE so
    # real transposes carry only their input-DMA wait (the transpose/LDW
    # instruction has a single sync-wait slot in walrus codegen).
    pt_warm = tp_psum.tile([P, P], F32, name="pt", tag="pt")
    nc.tensor.transpose(pt_warm, ident, ident)

    wg_f = const.tile([P, NH, E], F32)
    nc.sync.dma_start(wg_f, Wg.rearrange("(j p) e -> p j e", p=P))

    w2b = const.tile([P, E, HF], F32)
    nc.sync.dma_start(
        w2b, W2.rearrange("(o e) f -> o e f", o=1).partition_broadcast(P))

    if has_b1:
        b1b = const.tile([P, E, HF], F32)
        nc.sync.dma_start(
            b1b, b1.rearrange("(o e) f -> o e f", o=1).partition_broadcast(P))
    if has_bg:
        bgb = const.tile([P, E], F32)
        nc.sync.dma_start(
            bgb, bg.rearrange("(o e) -> o e", o=1).partition_broadcast(P))

    negbig = const.tile([P, E], F32)
    nc.vector.memset(negbig, NEG_BIG)

    # ---- persistent tiles ----
    xt = xtp.tile([P, NH, nb, P], MDT)        # x^T resident, bf16
    logit_sb = persist.tile([P, nb, E], F32)
    o_sb = persist.tile([P, nb, E], F32)
    fin_all = persist.tile([P, nb], F32)

    # ---- phase 1: stream x, transpose to xT, gating logits ----
    for b in range(nb):
        xn = xstage.tile([P, H], F32, name="xn")
        nc.sync.dma_start(xn, x[P * b:P * (b + 1), :])
        gp = g_psum.tile([E, P], F32, name="gp")
        xtfs = []
        for j in range(NH):
            pt = tp_psum.tile([P, P], F32, name="pt", tag="pt")
            nc.tensor.transpose(pt, xn[:, P * j:P * (j + 1)], ident)
            xtf = xstage.tile([P, P], F32, name="xtf", tag="xtf", bufs=NH + 2)
            nc.vector.tensor_copy(xtf, pt)
            nc.vector.tensor_copy(xt[:, j, b, :], xtf)
            xtfs.append(xtf)
        # fp32 gate matmuls with Wg as the 8-column stationary (LDWEIGHTS is
        # not hidden on this toolchain, so a 128-col fp32 stationary per
        # matmul costs ~350ns; an 8-col one is ~free). Output is logits^T
        # [8, 128]; transpose back through the PE. Full fp32 precision:
        # selection-critical logits (bf16 flips the 2nd/3rd expert choice
        # on ~30/16k tokens).
        for j in range(NH):
            nc.tensor.matmul(gp, lhsT=wg_f[:, j, :], rhs=xtfs[j],
                             start=(j == 0), stop=(j == NH - 1))
        lgT = xstage.tile([E, P], F32, name="lgT", tag="lgT", bufs=2)
        nc.vector.tensor_copy(lgT, gp)
        ptb = tp_psum.tile([P, E], F32, name="pt", tag="pt")
        nc.tensor.transpose(ptb, lgT, ident[0:E, 0:E])
        if has_bg:
            nc.vector.tensor_add(logit_sb[:, b, :], ptb, bgb)
        else:
            nc.vector.tensor_copy(logit_sb[:, b, :], ptb)

    if variant == "p1":
        nc.sync.dma_start(logits.rearrange("(bt p) e -> p bt e", p=P), logit_sb)
        return

    # ---- phase 2: experts, grouped ----
    for g in range(NG):
        w1bf = w1bfp.tile([P, NH, EG, HF], MDT, name="w1bf")
        for j in range(NH):
            st = w1stage.tile([P, EG, HF], F32, name="st")
            nc.sync.dma_start(
                st, W1[g * EG:(g + 1) * EG, P * j:P * (j + 1), :]
                .rearrange("e p f -> p e f"))
            nc.vector.tensor_copy(w1bf[:, j, :, :], st)
        for b in range(nb):
            pss = []
            for ee in range(EG):
                ps = mm_psum.tile([P, HF], F32, name="ps", tag="ps")
                pss.append(ps)
            for j in range(NH):
                for ee in range(EG):
                    nc.tensor.matmul(pss[ee], lhsT=xt[:, j, b, :],
                                     rhs=w1bf[:, j, ee, :],
                                     start=(j == 0), stop=(j == NH - 1))
            for ee in range(EG):
                e = g * EG + ee
                gl = gelup.tile([P, HF], F32, name="gl", tag="gl")
                if has_b1:
                    nc.vector.tensor_add(gl, pss[ee], b1b[:, e, :])
                    nc.scalar.activation(gl, gl, act_fn)
                else:
                    nc.scalar.activation(gl, pss[ee], act_fn)
                if variant == "p2a":
                    nc.vector.tensor_reduce(o_sb[:, b, e:e + 1], gl,
                                            axis=AXX, op=ALU.add)
                else:
                    prod2 = gelup.tile([P, HF], F32, name="prod2", tag="prod2")
                    nc.vector.tensor_mul(prod2, gl, w2b[:, e, :])
                    nc.vector.tensor_reduce(o_sb[:, b, e:e + 1], prod2,
                                            axis=AXX, op=ALU.add)

    if variant in ("p2", "p2a"):
        nc.sync.dma_start(logits.rearrange("(bt p) e -> p bt e", p=P), logit_sb)
        nc.sync.dma_start(scores.rearrange("(bt p) o -> p bt o", p=P),
                          o_sb[:, :, 0:1])
        return

    # ---- phase 3: top-2 softmax combine ----
    for b in range(nb):
        lg = logit_sb[:, b, :]
        m1 = small.tile([P, 1], F32, name="m1")
        nc.vector.tensor_reduce(m1, lg, axis=AXX, op=ALU.max)
        nm1 = small.tile([P, 1], F32, name="nm1")
        nc.vector.tensor_scalar_mul(nm1, m1, -1.0)
        eq = small.tile([P, E], mybir.dt.uint8, name="eq")
        nc.vector.tensor_scalar(eq, lg, m1, None, op0=ALU.is_equal)
        msk = small.tile([P, E], F32, name="msk")
        nc.vector.select(msk, eq, negbig, lg)
        m2 = small.tile([P, 1], F32, name="m2")
        nc.vector.tensor_reduce(m2, msk, axis=AXX, op=ALU.max)
        t = small.tile([P, E], F32, name="t")
        nc.scalar.activation(t, lg, EXP, bias=nm1, scale=1.0)
        keep = small.tile([P, E], F32, name="keep")
        nc.vector.tensor_scalar(keep, lg, m2, None, op0=ALU.is_ge)
        t2 = small.tile([P, E], F32, name="t2")
        nc.vector.tensor_mul(t2, t, keep)
        z = small.tile([P, 1], F32, name="z")
        nc.vector.tensor_reduce(z, t2, axis=AXX, op=ALU.add)
        s = small.tile([P, 1], F32, name="s")
        prod = small.tile([P, E], F32, name="prod")
        nc.vector.tensor_mul(prod, t2, o_sb[:, b, :])
        nc.vector.tensor_reduce(s, prod, axis=AXX, op=ALU.add)
        rz = small.tile([P, 1], F32, name="rz")
        nc.vector.reciprocal(rz, z)
        nc.vector.tensor_mul(fin_all[:, b:b + 1], s, rz)

    # ---- outputs ----
    nc.sync.dma_start(logits.rearrange("(bt p) e -> p bt e", p=P), logit_sb)
    nc.sync.dma_start(scores.rearrange("(bt p) o -> p bt o", p=P),
                      fin_all.rearrange("p (bt o) -> p bt o", o=1))


def build_nc(nb, has_b1, has_bg, act_fn=GELU, variant='full'):
    bl = nb * P
    nc = bacc.Bacc("TRN2", target_bir_lowering=False, debug=False)
    x = nc.declare_dram_parameter("x", [bl, H], F32, isOutput=False)
    W1 = nc.declare_dram_parameter("W1", [E, H, HF], F32, isOutput=False)
    b1 = nc.declare_dram_parameter("b1", [E, HF], F32, isOutput=False)
    W2 = nc.declare_dram_parameter("W2", [E, HF], F32, isOutput=False)
    Wg = nc.declare_dram_parameter("Wg", [H, E], F32, isOutput=False)
    bg = nc.declare_dram_parameter("bg", [E], F32, isOutput=False)
    scores = nc.declare_dram_parameter("scores", [bl, 1], F32, isOutput=True)
    logits = nc.declare_dram_parameter("logits", [bl, E], F32, isOutput=True)
    aps = (x[:, :], W1[:, :, :], b1[:, :], W2[:, :], Wg[:, :], bg[:],
           scores[:, :], logits[:, :])
    with tile.TileContext(nc) as tc:
        with ExitStack() as ctx:
            _emit(ctx, tc, aps, nb, has_b1, has_bg, act_fn, variant)
    nc.compile()
    return nc


_CACHE = {}


def _get_nc(nb, has_b1, has_bg):
    key = (nb, has_b1, has_bg)
    if key not in _CACHE:
        _CACHE[key] = build_nc(nb, has_b1, has_bg)
    return _CACHE[key]


def kernel(x, W1, b1, W2, Wg, bg, trace=False):
    x = np.ascontiguousarray(np.asarray(x, np.float32))
    W1 = np.ascontiguousarray(np.asarray(W1, np.float32))
    b1 = np.ascontiguousarray(np.asarray(b1, np.float32))
    W2 = np.ascontiguousarray(np.asarray(W2, np.float32))
    Wg = np.ascontiguousarray(np.asarray(Wg, np.float32))
    bg = np.ascontiguousarray(np.asarray(bg, np.float32))
    assert x.shape == (B, H)
    bl = B // NCORES
    nb = bl // P
    has_b1 = bool(np.any(b1))
    has_bg = bool(np.any(bg))
    nc = _get_nc(nb, has_b1, has_bg)
    in_maps = [
        {"x": x[c * bl:(c + 1) * bl], "W1": W1, "b1": b1, "W2": W2,
         "Wg": Wg, "bg": bg}
        for c in range(NCORES)
    ]
    res = run_bass_kernel_spmd(nc, in_maps, core_ids=list(range(NCORES)),
                               trace=trace)
    scores = np.concatenate([res.results[c]["scores"] for c in range(NCORES)],
                            axis=0)
    logits = np.concatenate([res.results[c]["logits"] for c in range(NCORES)],
                            axis=0)
    if trace:
        kernel.last_exec_time_ns = res.exec_time_ns
    return scores, logits


kernel.last_exec_time_ns = None
